# revision 19
# baseline (speedup 1.0000x reference)
"""GQA attention (B=1, T=2048, C=2048, 16 Q heads / 4 KV heads, head_dim=128)
with RoPE, logit softcap 50, causal mask, softmax, output projection.

Sharding: 16 Q-heads over 8 NeuronCores (2 Q-heads + their single KV head per
core). Each core computes its partial output projection over its 2 heads; the
host sums the 8 partials (the post-projection all-reduce).

Device layout (per core):
  xT  [C, T] bf16 in SBUF (C on partitions, 16 chunks)
  Q^T [k, s] per head, K^T [k, d]    from matmul(lhsT=W chunk, rhs=xT chunk)
  RoPE applied in [k, s] layout: rot(q) = Rm @ q via a sign-permutation matmul,
    then q*cosT + rot*sinT on VectorE.
  S^T [d, s] = matmul(lhsT=K^T block, rhs=Q^T chunk)  (so the post-softmax
    matrix is already the PV lhsT -> no transpose of P needed)
  softcap+mask+softmax: tanh on ScalarE (scale 1/(50*sqrt(128))), triangular
    -40 bias added on diagonal 128-blocks, exp on ScalarE (scale 50). Softcap
    bounds logits to +-50 so no max-subtraction is needed.
  PV: O_aug[s,129] = matmul(lhsT=P^T slice, rhs=V_aug) where V_aug has a ones
    column -> column 128 accumulates the softmax denominator for free.
  normalize by 1/r (per-partition scalar), transpose O via TensorE, output
    projection back to [s, m], DMA out f32.
"""

import sys

sys.path.insert(0, "/opt/trn_rl_repo")

import math
from contextlib import ExitStack

import numpy as np
import ml_dtypes

import concourse.bass as bass
import concourse.tile as tile
from concourse import bacc
from concourse import mybir
from concourse.bass_utils import run_bass_kernel_spmd
from concourse.masks import make_identity

BF16 = ml_dtypes.bfloat16
T = 2048
C = 2048
HD = 128
NQH, NKVH = 16, 4
R = NQH // NKVH  # 4
ROPE_THETA = 10000.0
SOFTCAP = 50.0
NCORES = 8

F32 = mybir.dt.float32
BF = mybir.dt.bfloat16
AFT = mybir.ActivationFunctionType

TANH_SCALE = 1.0 / (math.sqrt(float(HD)) * SOFTCAP)
MASK_BIAS = -40.0  # added to tanh output; exp scale 50 -> -2000 in the exponent

_NC_CACHE = {}


def build_nc():
    if "nc" in _NC_CACHE:
        return _NC_CACHE["nc"]
    nc = bacc.Bacc(None, target_bir_lowering=False)
    xT = nc.dram_tensor("xT", [C, T], BF, kind="ExternalInput")
    wq = nc.dram_tensor("wq", [C, 2 * HD], BF, kind="ExternalInput")
    wk = nc.dram_tensor("wk", [C, HD], BF, kind="ExternalInput")
    wv = nc.dram_tensor("wv", [C, HD], BF, kind="ExternalInput")
    wo = nc.dram_tensor("wo", [2 * HD, C], BF, kind="ExternalInput")
    cosT = nc.dram_tensor("cosT", [HD, T], BF, kind="ExternalInput")
    sinT = nc.dram_tensor("sinT", [HD, T], F32, kind="ExternalInput")
    rmT = nc.dram_tensor("rmT", [HD, HD], BF, kind="ExternalInput")
    tri = nc.dram_tensor("tri", [HD, HD], F32, kind="ExternalInput")
    out = nc.dram_tensor("out", [T, C], F32, kind="ExternalOutput")

    NCH = C // 128  # 16 contraction chunks
    NSB = T // 128  # 16 s-blocks
    NJ = T // 512  # 4 s-chunks of 512

    with tile.TileContext(nc) as tc, ExitStack() as ctx:
        consts = ctx.enter_context(tc.tile_pool(name="consts", bufs=1))
        qkv = ctx.enter_context(tc.tile_pool(name="qkv", bufs=1))
        osmall = ctx.enter_context(tc.tile_pool(name="osmall", bufs=2))
        outsb = ctx.enter_context(tc.tile_pool(name="outsb", bufs=2))
        tpool = ctx.enter_context(tc.tile_pool(name="tpool", bufs=2))
        ptpool = ctx.enter_context(tc.tile_pool(name="ptpool", bufs=1))
        # PSUM budget (8 banks): proj 2 + sg 4 + o 1 + ot 1
        ps = ctx.enter_context(tc.tile_pool(name="ps", bufs=2, space="PSUM"))
        ps_sg = ctx.enter_context(tc.tile_pool(name="ps_sg", bufs=2, space="PSUM"))
        ps_o = ctx.enter_context(tc.tile_pool(name="ps_o", bufs=1, space="PSUM"))
        ps_ot = ctx.enter_context(tc.tile_pool(name="ps_ot", bufs=1, space="PSUM"))

        ident = consts.tile([128, 128], BF, tag="ident")
        make_identity(nc, ident)
        tri_sb = consts.tile([128, 128], F32, tag="tri")
        nc.sync.dma_start(out=tri_sb, in_=tri[:, :])
        wo_sb = consts.tile([128, 2, C], BF, tag="wo")
        for h in range(2):
            nc.sync.dma_start(out=wo_sb[:, h, :], in_=wo[h * 128:(h + 1) * 128, :])

        QT = qkv.tile([128, 2, T], BF, tag="QT")
        KT = qkv.tile([128, T], BF, tag="KT")
        Vaug = qkv.tile([128, NCH, 132], BF, tag="Vaug")
        OT = qkv.tile([128, 2, T], BF, tag="OT")
        nc.vector.memset(Vaug[:, :, 128:129], 1.0)

        pt_tiles = {}

        def attn_scores(J, i_lo=0, i_hi=None):
            n_i = 4 * J + 4
            if i_hi is None:
                i_hi = n_i
            if i_lo == 0:
                PT = ptpool.tile(
                    [128, 2, NSB, 512], BF, tag="pt", name=f"PT{J}"
                )
                pt_tiles[J] = PT
            else:
                PT = pt_tiles[J]
            for i in range(i_lo, i_hi):
                b = i - 4 * J
                c0 = 256 if b >= 2 else 0  # cols below are never consumed
                csl = slice(c0, 512)
                sg = ps_sg.tile([128, 2, 512], F32, tag="sg")
                for h in range(2):
                    nc.tensor.matmul(
                        sg[:, h, csl],
                        KT[:, i * 128:(i + 1) * 128],
                        QT[:, h, J * 512 + c0:(J + 1) * 512],
                        start=True, stop=True,
                    )
                tt = tpool.tile([128, 2, 512], F32, tag="t")
                nc.scalar.activation(
                    tt[:, :, csl], sg[:, :, csl], AFT.Tanh, scale=TANH_SCALE
                )
                if b >= 0:  # diagonal block: apply triangular mask bias
                    dsl = slice(b * 128, (b + 1) * 128)
                    for h in range(2):
                        nc.vector.tensor_add(tt[:, h, dsl], tt[:, h, dsl], tri_sb)
                nc.scalar.activation(
                    PT[:, :, i, csl], tt[:, :, csl], AFT.Exp, scale=SOFTCAP
                )

        def attn_pv_out(J, sbs=(0, 1, 2, 3), pop=True):
            PT = pt_tiles.pop(J) if pop else pt_tiles[J]
            for sb_ in sbs:
                j = 4 * J + sb_
                for h in range(2):
                    po = ps_o.tile([128, 129], F32, tag="o")
                    for i in range(j + 1):
                        nc.tensor.matmul(
                            po,
                            PT[:, h, i, sb_ * 128:(sb_ + 1) * 128],
                            Vaug[:, i, 0:129],
                            start=(i == 0), stop=(i == j),
                        )
                    rinv = osmall.tile([128, 1], F32, tag="rinv")
                    nc.vector.reciprocal(rinv, po[:, 128:129])
                    on = osmall.tile([128, 128], BF, tag="on")
                    nc.vector.tensor_scalar_mul(on, po[:, 0:128], rinv)
                    pot = ps_ot.tile([128, 128], BF, tag="ot")
                    nc.tensor.transpose(pot, on, ident)
                    nc.vector.tensor_copy(OT[:, h, j * 128:(j + 1) * 128], pot)
                # fused output projection for this s-block; ldweights of
                # OT[h] shared across an m-chunk pair; one 1MB DMA per block
                ob = outsb.tile([128, T], F32, tag="ob")
                for mg in range(2):
                    pp = [ps.tile([128, 512], F32, tag="proj", name=f"po{j}_{mg}{_i}")
                          for _i in range(2)]
                    for h in range(2):
                        for pi in range(2):
                            mch = 2 * mg + pi
                            nc.tensor.matmul(
                                pp[pi],
                                OT[:, h, j * 128:(j + 1) * 128],
                                wo_sb[:, h, mch * 512:(mch + 1) * 512],
                                start=(h == 0), stop=(h == 1),
                            )
                    for pi in range(2):
                        mch = 2 * mg + pi
                        nc.vector.tensor_copy(
                            ob[:, mch * 512:(mch + 1) * 512], pp[pi]
                        )
                nc.sync.dma_start(out=out[j * 128:(j + 1) * 128, :], in_=ob)

        with tc.tile_pool(name="ph1", bufs=1) as ph1, \
             tc.tile_pool(name="work", bufs=3) as work, \
             tc.tile_pool(name="ropet", bufs=2) as ropet:
            rm_sb = ph1.tile([128, 128], BF, tag="rm")
            cos_sb = ph1.tile([128, T], BF, tag="cos")
            sin_sb = ph1.tile([128, T], F32, tag="sin")
            wq_sb = ph1.tile([128, NCH, 2 * HD], BF, tag="wq")
            wk_sb = ph1.tile([128, NCH, HD], BF, tag="wk")
            wv_sb = ph1.tile([128, NCH, HD], BF, tag="wv")
            x_sb = ph1.tile([128, NCH, T], BF, tag="x")
            # batched DMAs (DMA_DIRECT2D issue is ~600ns each on Sync):
            # weights first (small), x staggered so K matmuls start early.
            def dma_chunks(dst, src, lo, hi):
                nc.sync.dma_start(
                    out=dst[:, lo:hi, :],
                    in_=src.rearrange("(c p) s -> p c s", p=128)[:, lo:hi, :],
                )
            dma_chunks(wk_sb, wk, 0, NCH)
            dma_chunks(x_sb, xT, 0, 2)
            dma_chunks(x_sb, xT, 2, 4)
            dma_chunks(wq_sb, wq, 0, NCH)
            dma_chunks(x_sb, xT, 4, 8)
            nc.sync.dma_start(out=rm_sb, in_=rmT[:, :])
            nc.sync.dma_start(out=cos_sb, in_=cosT[:, :])
            nc.sync.dma_start(out=sin_sb, in_=sinT[:, :])
            dma_chunks(wv_sb, wv, 0, NCH)
            dma_chunks(x_sb, xT, 8, 12)
            dma_chunks(x_sb, xT, 12, 16)

            def rope_chunk(z, ch, dst):
                sl = slice(ch * 512, (ch + 1) * 512)
                pr = ps.tile([128, 512], F32, tag="proj")
                nc.tensor.matmul(pr, rm_sb, z, start=True, stop=True)
                m2 = ropet.tile([128, 512], F32, tag="m2")
                nc.vector.tensor_mul(m2, pr, sin_sb[:, sl])
                m1 = ropet.tile([128, 512], F32, tag="m1")
                nc.vector.tensor_mul(m1, z, cos_sb[:, sl])
                nc.vector.tensor_add(dst[:, sl], m1, m2)

            def proj_chunk(w_slice_fn, ch, dst):
                sl = slice(ch * 512, (ch + 1) * 512)
                p = ps.tile([128, 512], F32, tag="proj")
                for c in range(NCH):
                    nc.tensor.matmul(
                        p, w_slice_fn(c), x_sb[:, c, sl],
                        start=(c == 0), stop=(c == NCH - 1),
                    )
                z = work.tile([128, 512], BF, tag="z")
                nc.scalar.copy(z, p)
                rope_chunk(z, ch, dst)

            def v_chunk(ch):
                sl = slice(ch * 512, (ch + 1) * 512)
                p = ps.tile([128, 512], F32, tag="proj")
                for c in range(NCH):
                    nc.tensor.matmul(
                        p, wv_sb[:, c, :], x_sb[:, c, sl],
                        start=(c == 0), stop=(c == NCH - 1),
                    )
                z = work.tile([128, 512], BF, tag="z")
                nc.scalar.copy(z, p)
                for b in range(4):
                    dt = 4 * ch + b
                    pv = ps_ot.tile([128, 128], BF, tag="ot")
                    nc.tensor.transpose(pv, z[:, b * 128:(b + 1) * 128], ident)
                    nc.vector.tensor_copy(Vaug[:, dt, 0:128], pv)

            # K: c-outer accumulation (borrows the two sg slots) -- matmuls
            # start with the first streamed x quarter, ldweights amortized.
            k0 = work.tile([128, T], BF, tag="zk", bufs=1)
            pk = [ps_sg.tile([128, 2, 512], F32, tag="sg", name=f"pk{_i}")
                  for _i in range(2)]
            for c in range(NCH):
                for ch in range(NJ):
                    nc.tensor.matmul(
                        pk[ch // 2][:, ch % 2, :],
                        wk_sb[:, c, :],
                        x_sb[:, c, ch * 512:(ch + 1) * 512],
                        start=(c == 0), stop=(c == NCH - 1),
                    )
            for half in range(2):
                nc.scalar.copy(
                    k0[:, half * 1024:(half + 1) * 1024].rearrange(
                        "p (a b) -> p a b", a=2
                    ),
                    pk[half],
                )
            for ch in range(NJ):
                rope_chunk(k0[:, ch * 512:(ch + 1) * 512], ch, KT)

            # round pipeline: per 512-chunk J, PV of J-1 first (frees the PT
            # slot while the Q chunks project), then Q chunks, scores, V chunk
            for ch in range(NJ):
                if ch >= 1:
                    attn_pv_out(ch - 1)
                proj_chunk(lambda c: wq_sb[:, c, 0:HD], ch, QT[:, 0, :])
                proj_chunk(lambda c: wq_sb[:, c, HD:2 * HD], ch, QT[:, 1, :])
                if ch < 3:
                    attn_scores(ch)
                    v_chunk(ch)
                else:
                    attn_scores(3, 0, 13)
                    v_chunk(3)
                    attn_pv_out(3, sbs=(0,), pop=False)
                    attn_scores(3, 13, 14)
                    attn_pv_out(3, sbs=(1,), pop=False)
                    attn_scores(3, 14, 15)
                    attn_pv_out(3, sbs=(2,), pop=False)
                    attn_scores(3, 15, 16)
                    attn_pv_out(3, sbs=(3,))

    nc.finalize()
    _NC_CACHE["nc"] = nc
    return nc


def _rope_tables():
    fraction = np.arange(0, HD, 2, dtype=np.float64) / HD
    timescale = ROPE_THETA ** fraction
    inv = 1.0 / timescale
    sin_inp = np.outer(np.arange(T, dtype=np.float64), inv)
    sin_inp = np.concatenate([sin_inp, sin_inp], axis=-1)  # [T, HD]
    sin = np.sin(sin_inp).astype(np.float32)
    cos = np.cos(sin_inp).astype(np.float32)
    return cos.T.copy(), sin.T.copy()  # [HD, T]


def _numpy_fallback(x, mask, q_kernel, k_kernel, v_kernel, out_kernel):
    # generic-mask reference path (host, f32) - only used if the mask is not
    # the standard causal mask.
    b, t, c = x.shape
    q = np.einsum("bsm,mrhk->brhsk", x, q_kernel)
    k = np.einsum("bdm,mhk->bhdk", x, k_kernel)
    v = np.einsum("bdm,mhv->bhdv", x, v_kernel)
    cosT, sinT = _rope_tables()
    cos, sin = cosT.T, sinT.T  # [T, HD]

    def rot(z):
        z1, z2 = np.split(z, 2, axis=-1)
        return np.concatenate([-z2, z1], axis=-1)

    q = q * cos[None, None, None] + rot(q) * sin[None, None, None]
    k = k * cos[None, None] + rot(k) * sin[None, None]
    s = np.einsum("brhsk,bhdk->brhsd", q, k) / np.sqrt(np.float32(HD))
    s = np.tanh(s / SOFTCAP) * SOFTCAP
    m = mask[:, None]  # [B,1,1,T,T]
    s = np.where(m, s, -np.inf)
    s = s - s.max(axis=-1, keepdims=True)
    e = np.exp(s)
    p = e / e.sum(axis=-1, keepdims=True)
    p = np.where(m, p, 0.0)
    qkv = np.einsum("brhsd,bhdv->brhsv", p, v)
    return np.einsum("brhsv,rhvm->bsm", qkv, out_kernel).astype(np.float32)


def kernel(x, mask, q_kernel, k_kernel, v_kernel, out_kernel, _trace=False):
    x = np.asarray(x)
    mask = np.asarray(mask)
    causal = bool(
        np.array_equal(mask[0, 0], np.tril(np.ones((T, T), dtype=bool)))
    )
    if not causal:
        return _numpy_fallback(x, mask, q_kernel, k_kernel, v_kernel, out_kernel)

    q_kernel = np.asarray(q_kernel, dtype=np.float32)
    k_kernel = np.asarray(k_kernel, dtype=np.float32)
    v_kernel = np.asarray(v_kernel, dtype=np.float32)
    out_kernel = np.asarray(out_kernel, dtype=np.float32)

    xT = np.ascontiguousarray(x[0].T).astype(BF16)
    cosT, sinT = _rope_tables()
    cosT_bf = cosT.astype(BF16)
    rm = np.zeros((HD, HD), dtype=np.float32)
    for kk in range(HD // 2):
        rm[kk, kk + HD // 2] = -1.0
    for kk in range(HD // 2, HD):
        rm[kk, kk - HD // 2] = 1.0
    rmT = np.ascontiguousarray(rm.T).astype(BF16)
    dl = np.arange(128)[:, None]
    sl = np.arange(128)[None, :]
    tri = np.where(dl <= sl, 0.0, MASK_BIAS).astype(np.float32)

    in_maps = []
    for core in range(NCORES):
        h = core // 2
        r0 = (core % 2) * 2
        wq_c = np.ascontiguousarray(
            q_kernel[:, r0:r0 + 2, h, :].reshape(C, 2 * HD)
        ).astype(BF16)
        wk_c = np.ascontiguousarray(k_kernel[:, h, :]).astype(BF16)
        wv_c = np.ascontiguousarray(v_kernel[:, h, :]).astype(BF16)
        wo_c = np.ascontiguousarray(
            out_kernel[r0:r0 + 2, h, :, :].reshape(2 * HD, C)
        ).astype(BF16)
        in_maps.append({
            "xT": xT, "wq": wq_c, "wk": wk_c, "wv": wv_c, "wo": wo_c,
            "cosT": cosT_bf, "sinT": sinT, "rmT": rmT, "tri": tri,
        })

    nc = build_nc()
    res = run_bass_kernel_spmd(
        nc, in_maps, core_ids=list(range(NCORES)), trace=_trace
    )
    total = np.zeros((T, C), dtype=np.float32)
    for om in res.results:
        total += om["out"]
    out = total[None]
    if _trace:
        return out, res
    return out


# revision 24
# speedup vs baseline: 1.3618x; 1.3618x over previous
"""GQA attention (B=1, T=2048, C=2048, 16 Q heads / 4 KV heads, head_dim=128)
with RoPE, logit softcap 50, causal mask, softmax, output projection.

Sharding: 16 Q-heads over 8 NeuronCores (2 Q-heads + their single KV head per
core). Each core computes its partial output projection over its 2 heads; the
host sums the 8 partials (the post-projection all-reduce).

Device layout (per core):
  xT  [C, T] bf16 in SBUF (C on partitions, 16 chunks)
  Q^T [k, s] per head, K^T [k, d]    from matmul(lhsT=W chunk, rhs=xT chunk)
  RoPE applied in [k, s] layout: rot(q) = Rm @ q via a sign-permutation matmul,
    then q*cosT + rot*sinT on VectorE.
  S^T [d, s] = matmul(lhsT=K^T block, rhs=Q^T chunk)  (so the post-softmax
    matrix is already the PV lhsT -> no transpose of P needed)
  softcap+mask+softmax: tanh on ScalarE (scale 1/(50*sqrt(128))), triangular
    -40 bias added on diagonal 128-blocks, exp on ScalarE (scale 50). Softcap
    bounds logits to +-50 so no max-subtraction is needed.
  PV: O_aug[s,129] = matmul(lhsT=P^T slice, rhs=V_aug) where V_aug has a ones
    column -> column 128 accumulates the softmax denominator for free.
  normalize by 1/r (per-partition scalar), transpose O via TensorE, output
    projection back to [s, m], DMA out f32.
"""

import sys

sys.path.insert(0, "/opt/trn_rl_repo")

import math
from contextlib import ExitStack

import numpy as np
import ml_dtypes

import concourse.bass as bass
import concourse.tile as tile
from concourse.masks import make_identity
from concourse import bacc
from concourse import mybir
from concourse.bass_utils import run_bass_kernel_spmd

BF16 = ml_dtypes.bfloat16
T = 2048
C = 2048
HD = 128
NQH, NKVH = 16, 4
R = NQH // NKVH  # 4
ROPE_THETA = 10000.0
SOFTCAP = 50.0
NCORES = 8

F32 = mybir.dt.float32
BF = mybir.dt.bfloat16
AFT = mybir.ActivationFunctionType

TANH_SCALE = 1.0 / (math.sqrt(float(HD)) * SOFTCAP)
MASK_BIAS = -40.0  # added to tanh output; exp scale 50 -> -2000 in the exponent

_NC_CACHE = {}


def build_nc():
    if "nc" in _NC_CACHE:
        return _NC_CACHE["nc"]
    nc = bacc.Bacc(None, target_bir_lowering=False)
    xT = nc.dram_tensor("xT", [C, T], BF, kind="ExternalInput")
    wq = nc.dram_tensor("wq", [C, 2 * HD], BF, kind="ExternalInput")
    wk = nc.dram_tensor("wk", [C, HD], BF, kind="ExternalInput")
    wv = nc.dram_tensor("wv", [C, HD], BF, kind="ExternalInput")
    wo = nc.dram_tensor("wo", [2 * HD, C], BF, kind="ExternalInput")
    cosT = nc.dram_tensor("cosT", [HD, T], BF, kind="ExternalInput")
    sinT = nc.dram_tensor("sinT", [HD, T], F32, kind="ExternalInput")
    rmT = nc.dram_tensor("rmT", [HD, HD], BF, kind="ExternalInput")
    tri = nc.dram_tensor("tri", [HD, HD], F32, kind="ExternalInput")
    out = nc.dram_tensor("out", [T, C], F32, kind="ExternalOutput")

    NCH = C // 128  # 16 contraction chunks
    NSB = T // 128  # 16 s-blocks
    NJ = T // 512  # 4 s-chunks of 512

    with tile.TileContext(nc) as tc, ExitStack() as ctx:
        consts = ctx.enter_context(tc.tile_pool(name="consts", bufs=1))
        qkv = ctx.enter_context(tc.tile_pool(name="qkv", bufs=1))
        osmall = ctx.enter_context(tc.tile_pool(name="osmall", bufs=2))
        outsb = ctx.enter_context(tc.tile_pool(name="outsb", bufs=2))
        tpool = ctx.enter_context(tc.tile_pool(name="tpool", bufs=2))
        ptpool = []
        # PSUM budget (8 banks): proj 2 + sg 4 + o 1 + ot 1
        ps = ctx.enter_context(tc.tile_pool(name="ps", bufs=3, space="PSUM"))
        ps_sg = ctx.enter_context(tc.tile_pool(name="ps_sg", bufs=2, space="PSUM"))
        ps_ot = ctx.enter_context(tc.tile_pool(name="ps_ot", bufs=1, space="PSUM"))

        ident = consts.tile([128, 128], BF, tag="ident")
        make_identity(nc, ident)
        tri_sb = consts.tile([128, 128], F32, tag="tri")
        nc.sync.dma_start(out=tri_sb, in_=tri[:, :])
        wo_sb = consts.tile([128, 2, C], BF, tag="wo")
        for h in range(2):
            nc.sync.dma_start(out=wo_sb[:, h, :], in_=wo[h * 128:(h + 1) * 128, :])

        QT = qkv.tile([128, 2, T], BF, tag="QT")
        KT = qkv.tile([128, T], BF, tag="KT")
        Vaug = qkv.tile([128, NCH, 132], BF, tag="Vaug")
        OT = qkv.tile([128, 2, T], BF, tag="OT")
        nc.vector.memset(Vaug[:, :, 128:129], 1.0)

        pt_tiles = {}

        def attn_scores(J, i_lo=0, i_hi=None):
            n_i = 4 * J + 4
            if i_hi is None:
                i_hi = n_i
            if i_lo == 0:
                pool_ = qkv if J < 2 else ptpool[0]
                PT = pool_.tile(
                    [128, 2, n_i, 512], BF, tag=f"pt{min(J, 2)}", name=f"PT{J}"
                )
                pt_tiles[J] = PT
            else:
                PT = pt_tiles[J]
            for i in range(i_lo, i_hi):
                b = i - 4 * J
                c0 = 256 if b >= 2 else 0  # cols below are never consumed
                csl = slice(c0, 512)
                sg = ps_sg.tile([128, 2, 512], F32, tag="sg")
                for h in range(2):
                    nc.tensor.matmul(
                        sg[:, h, csl],
                        KT[:, i * 128:(i + 1) * 128],
                        QT[:, h, J * 512 + c0:(J + 1) * 512],
                        start=True, stop=True,
                    )
                tt = tpool.tile([128, 2, 512], F32, tag="t")
                nc.scalar.activation(
                    tt[:, :, csl], sg[:, :, csl], AFT.Tanh, scale=TANH_SCALE
                )
                if b >= 0:  # diagonal block: apply triangular mask bias
                    dsl = slice(b * 128, (b + 1) * 128)
                    for h in range(2):
                        nc.vector.tensor_add(tt[:, h, dsl], tt[:, h, dsl], tri_sb)
                nc.scalar.activation(
                    PT[:, :, i, csl], tt[:, :, csl], AFT.Exp, scale=SOFTCAP
                )

        def attn_pv_out(J, sbs=(0, 1, 2, 3), pop=True):
            PT = pt_tiles.pop(J) if pop else pt_tiles[J]
            for sb_ in sbs:
                j = 4 * J + sb_
                for h in range(2):
                    po = ps.tile([128, 512], F32, tag="proj", name=f"po_{J}_{sb_}_{h}")
                    for i in range(j + 1):
                        nc.tensor.matmul(
                            po[:, 0:129],
                            PT[:, h, i, sb_ * 128:(sb_ + 1) * 128],
                            Vaug[:, i, 0:129],
                            start=(i == 0), stop=(i == j),
                        )
                    rinv = osmall.tile([128, 1], F32, tag="rinv")
                    nc.vector.reciprocal(rinv, po[:, 128:129])
                    on = osmall.tile([128, 128], BF, tag="on")
                    nc.vector.tensor_scalar_mul(on, po[:, 0:128], rinv)
                    pot = ps_ot.tile([128, 128], BF, tag="ot")
                    nc.tensor.transpose(pot, on, ident)
                    nc.vector.tensor_copy(OT[:, h, j * 128:(j + 1) * 128], pot)
                # fused output projection for this s-block; ldweights of
                # OT[h] shared across an m-chunk pair; one 1MB DMA per block
                ob = outsb.tile([128, T], F32, tag="ob")
                for mg in range(2):
                    pp = [ps.tile([128, 512], F32, tag="proj", name=f"po{j}_{mg}{_i}")
                          for _i in range(2)]
                    for h in range(2):
                        for pi in range(2):
                            mch = 2 * mg + pi
                            nc.tensor.matmul(
                                pp[pi],
                                OT[:, h, j * 128:(j + 1) * 128],
                                wo_sb[:, h, mch * 512:(mch + 1) * 512],
                                start=(h == 0), stop=(h == 1),
                            )
                    for pi in range(2):
                        mch = 2 * mg + pi
                        nc.vector.tensor_copy(
                            ob[:, mch * 512:(mch + 1) * 512], pp[pi]
                        )
                nc.sync.dma_start(out=out[j * 128:(j + 1) * 128, :], in_=ob)

        with tc.tile_pool(name="ph1", bufs=1) as ph1, \
             tc.tile_pool(name="work", bufs=3) as work, \
             tc.tile_pool(name="ropet", bufs=2) as ropet:
            rm_sb = ph1.tile([128, 128], BF, tag="rm")
            cos_sb = ph1.tile([128, T], BF, tag="cos")
            sin_sb = ph1.tile([128, T], F32, tag="sin")
            wq_sb = ph1.tile([128, NCH, 2 * HD], BF, tag="wq")
            wk_sb = ph1.tile([128, NCH, HD], BF, tag="wk")
            wv_sb = ph1.tile([128, NCH, HD], BF, tag="wv")
            x_sb = ph1.tile([128, NCH, T], BF, tag="x")
            # batched DMAs (DMA_DIRECT2D issue is ~600ns each on Sync):
            # weights first (small), x staggered so K matmuls start early.
            def dma_chunks(dst, src, lo, hi):
                nc.sync.dma_start(
                    out=dst[:, lo:hi, :],
                    in_=src.rearrange("(c p) s -> p c s", p=128)[:, lo:hi, :],
                )
            dma_chunks(wk_sb, wk, 0, NCH)
            dma_chunks(x_sb, xT, 0, 2)
            dma_chunks(x_sb, xT, 2, 4)
            dma_chunks(wq_sb, wq, 0, NCH)
            dma_chunks(x_sb, xT, 4, 8)
            nc.sync.dma_start(out=rm_sb, in_=rmT[:, :])
            nc.sync.dma_start(out=cos_sb, in_=cosT[:, :])
            nc.sync.dma_start(out=sin_sb, in_=sinT[:, :])
            dma_chunks(wv_sb, wv, 0, NCH)
            dma_chunks(x_sb, xT, 8, 12)
            dma_chunks(x_sb, xT, 12, 16)

            def rope_chunk(z, ch, dst):
                sl = slice(ch * 512, (ch + 1) * 512)
                pr = ps.tile([128, 512], F32, tag="proj")
                nc.tensor.matmul(pr, rm_sb, z, start=True, stop=True)
                m2 = ropet.tile([128, 512], F32, tag="m2")
                nc.vector.tensor_mul(m2, pr, sin_sb[:, sl])
                m1 = ropet.tile([128, 512], F32, tag="m1")
                nc.vector.tensor_mul(m1, z, cos_sb[:, sl])
                nc.vector.tensor_add(dst[:, sl], m1, m2)

            def proj_chunk(w_slice_fn, ch, dst):
                sl = slice(ch * 512, (ch + 1) * 512)
                p = ps.tile([128, 512], F32, tag="proj")
                for c in range(NCH):
                    nc.tensor.matmul(
                        p, w_slice_fn(c), x_sb[:, c, sl],
                        start=(c == 0), stop=(c == NCH - 1),
                    )
                z = work.tile([128, 512], BF, tag="z")
                nc.scalar.copy(z, p)
                rope_chunk(z, ch, dst)

            def v_chunk(ch):
                sl = slice(ch * 512, (ch + 1) * 512)
                p = ps.tile([128, 512], F32, tag="proj")
                for c in range(NCH):
                    nc.tensor.matmul(
                        p, wv_sb[:, c, :], x_sb[:, c, sl],
                        start=(c == 0), stop=(c == NCH - 1),
                    )
                z = work.tile([128, 512], BF, tag="z")
                nc.scalar.copy(z, p)
                for b in range(4):
                    dt = 4 * ch + b
                    pv = ps_ot.tile([128, 128], BF, tag="ot")
                    nc.tensor.transpose(pv, z[:, b * 128:(b + 1) * 128], ident)
                    nc.vector.tensor_copy(Vaug[:, dt, 0:128], pv)

            # K: c-outer accumulation (borrows the two sg slots) -- matmuls
            # start with the first streamed x quarter, ldweights amortized.
            # K (all 4 chunks, sg slots) + Q0 (first 2 chunks, proj slots)
            # accumulate together while x streams in, keeping PE fed at the
            # DMA arrival rate.
            k0 = work.tile([128, T], BF, tag="zk", bufs=2)
            q0 = work.tile([128, T], BF, tag="zk", bufs=2)
            pk = [ps_sg.tile([128, 2, 512], F32, tag="sg", name=f"pk{_i}")
                  for _i in range(2)]
            pq = [ps.tile([128, 512], F32, tag="proj", name=f"pq{_i}")
                  for _i in range(2)]
            for c in range(NCH):
                for ch in range(NJ):
                    nc.tensor.matmul(
                        pk[ch // 2][:, ch % 2, :],
                        wk_sb[:, c, :],
                        x_sb[:, c, ch * 512:(ch + 1) * 512],
                        start=(c == 0), stop=(c == NCH - 1),
                    )
                for ch in range(2):
                    nc.tensor.matmul(
                        pq[ch],
                        wq_sb[:, c, 0:HD],
                        x_sb[:, c, ch * 512:(ch + 1) * 512],
                        start=(c == 0), stop=(c == NCH - 1),
                    )
            for half in range(2):
                nc.scalar.copy(
                    k0[:, half * 1024:(half + 1) * 1024].rearrange(
                        "p (a b) -> p a b", a=2
                    ),
                    pk[half],
                )
            for ch in range(2):
                nc.scalar.copy(q0[:, ch * 512:(ch + 1) * 512], pq[ch])
            for ch in range(NJ):
                rope_chunk(k0[:, ch * 512:(ch + 1) * 512], ch, KT)
            for ch in range(2):
                rope_chunk(q0[:, ch * 512:(ch + 1) * 512], ch, QT[:, 0, :])
            for ch in range(2, NJ):
                proj_chunk(lambda c: wq_sb[:, c, 0:HD], ch, QT[:, 0, :])
            for ch in range(NJ):
                proj_chunk(lambda c: wq_sb[:, c, HD:2 * HD], ch, QT[:, 1, :])

            attn_scores(0)
            attn_scores(1)

            for ch in range(NJ):
                v_chunk(ch)

        ptpool.append(ctx.enter_context(tc.tile_pool(name="ptpool", bufs=2)))
        attn_pv_out(0)
        attn_scores(2)
        attn_pv_out(1)
        attn_scores(3, 0, 13)
        attn_pv_out(2)
        attn_pv_out(3, sbs=(0,), pop=False)
        attn_scores(3, 13, 14)
        attn_pv_out(3, sbs=(1,), pop=False)
        attn_scores(3, 14, 15)
        attn_pv_out(3, sbs=(2,), pop=False)
        attn_scores(3, 15, 16)
        attn_pv_out(3, sbs=(3,))

    nc.finalize()
    _NC_CACHE["nc"] = nc
    return nc


def _rope_tables():
    fraction = np.arange(0, HD, 2, dtype=np.float64) / HD
    timescale = ROPE_THETA ** fraction
    inv = 1.0 / timescale
    sin_inp = np.outer(np.arange(T, dtype=np.float64), inv)
    sin_inp = np.concatenate([sin_inp, sin_inp], axis=-1)  # [T, HD]
    sin = np.sin(sin_inp).astype(np.float32)
    cos = np.cos(sin_inp).astype(np.float32)
    return cos.T.copy(), sin.T.copy()  # [HD, T]


def _numpy_fallback(x, mask, q_kernel, k_kernel, v_kernel, out_kernel):
    # generic-mask reference path (host, f32) - only used if the mask is not
    # the standard causal mask.
    b, t, c = x.shape
    q = np.einsum("bsm,mrhk->brhsk", x, q_kernel)
    k = np.einsum("bdm,mhk->bhdk", x, k_kernel)
    v = np.einsum("bdm,mhv->bhdv", x, v_kernel)
    cosT, sinT = _rope_tables()
    cos, sin = cosT.T, sinT.T  # [T, HD]

    def rot(z):
        z1, z2 = np.split(z, 2, axis=-1)
        return np.concatenate([-z2, z1], axis=-1)

    q = q * cos[None, None, None] + rot(q) * sin[None, None, None]
    k = k * cos[None, None] + rot(k) * sin[None, None]
    s = np.einsum("brhsk,bhdk->brhsd", q, k) / np.sqrt(np.float32(HD))
    s = np.tanh(s / SOFTCAP) * SOFTCAP
    m = mask[:, None]  # [B,1,1,T,T]
    s = np.where(m, s, -np.inf)
    s = s - s.max(axis=-1, keepdims=True)
    e = np.exp(s)
    p = e / e.sum(axis=-1, keepdims=True)
    p = np.where(m, p, 0.0)
    qkv = np.einsum("brhsd,bhdv->brhsv", p, v)
    return np.einsum("brhsv,rhvm->bsm", qkv, out_kernel).astype(np.float32)


def kernel(x, mask, q_kernel, k_kernel, v_kernel, out_kernel, _trace=False):
    x = np.asarray(x)
    mask = np.asarray(mask)
    causal = bool(
        np.array_equal(mask[0, 0], np.tril(np.ones((T, T), dtype=bool)))
    )
    if not causal:
        return _numpy_fallback(x, mask, q_kernel, k_kernel, v_kernel, out_kernel)

    q_kernel = np.asarray(q_kernel, dtype=np.float32)
    k_kernel = np.asarray(k_kernel, dtype=np.float32)
    v_kernel = np.asarray(v_kernel, dtype=np.float32)
    out_kernel = np.asarray(out_kernel, dtype=np.float32)

    xT = np.ascontiguousarray(x[0].T).astype(BF16)
    cosT, sinT = _rope_tables()
    cosT_bf = cosT.astype(BF16)
    rm = np.zeros((HD, HD), dtype=np.float32)
    for kk in range(HD // 2):
        rm[kk, kk + HD // 2] = -1.0
    for kk in range(HD // 2, HD):
        rm[kk, kk - HD // 2] = 1.0
    rmT = np.ascontiguousarray(rm.T).astype(BF16)
    dl = np.arange(128)[:, None]
    sl = np.arange(128)[None, :]
    tri = np.where(dl <= sl, 0.0, MASK_BIAS).astype(np.float32)

    in_maps = []
    for core in range(NCORES):
        h = core // 2
        r0 = (core % 2) * 2
        wq_c = np.ascontiguousarray(
            q_kernel[:, r0:r0 + 2, h, :].reshape(C, 2 * HD)
        ).astype(BF16)
        wk_c = np.ascontiguousarray(k_kernel[:, h, :]).astype(BF16)
        wv_c = np.ascontiguousarray(v_kernel[:, h, :]).astype(BF16)
        wo_c = np.ascontiguousarray(
            out_kernel[r0:r0 + 2, h, :, :].reshape(2 * HD, C)
        ).astype(BF16)
        in_maps.append({
            "xT": xT, "wq": wq_c, "wk": wk_c, "wv": wv_c, "wo": wo_c,
            "cosT": cosT_bf, "sinT": sinT, "rmT": rmT, "tri": tri,
        })

    nc = build_nc()
    res = run_bass_kernel_spmd(
        nc, in_maps, core_ids=list(range(NCORES)), trace=_trace
    )
    total = np.zeros((T, C), dtype=np.float32)
    for om in res.results:
        total += om["out"]
    out = total[None]
    if _trace:
        return out, res
    return out


# revision 29
# speedup vs baseline: 1.4001x; 1.0281x over previous
"""GQA attention (B=1, T=2048, C=2048, 16 Q heads / 4 KV heads, head_dim=128)
with RoPE, logit softcap 50, causal mask, softmax, output projection.

Sharding: 16 Q-heads over 8 NeuronCores (2 Q-heads + their single KV head per
core, tensor-parallel over the kv-head axis per the sharding hint). Each core
computes its partial output projection over its 2 heads; the host sums the 8
partials (the post-projection all-reduce).

Per-core device kernel (all matmuls bf16 with f32 PSUM accumulation):
  xT [C, T] bf16 resident in SBUF; K^T/Q^T projected in [head_dim, seq]
  layout via matmul(lhsT=W chunk, rhs=xT chunk). The K projection (plus the
  first half of Q0) accumulates c-outer while x streams in, so PE starts at
  the first 2MB DMA quarter. DMAs are batched (issue costs ~600ns each).
  RoPE in [k, s] layout: rot(q) = Rm @ q via a sign-permutation matmul, then
  q*cosT + rot*sinT on VectorE.
  S^T [d, s] = matmul(lhsT=K^T block, rhs=Q^T chunk): the post-softmax matrix
  is then already the PV lhsT -- no transpose of P. Both heads share each
  K-block ldweights. 512-granular causal: blocks entirely above the diagonal
  are skipped; partially-masked right-of-diagonal columns are trimmed.
  Softcap bounds logits to +-50, so softmax needs no max pass: tanh on
  ScalarE (scale 1/(50*sqrt(128))), triangular -40 bias on diagonal blocks
  (DVE), exp on ScalarE (scale 50; tanh+exp share one ACT table set).
  PV: O_aug[s, 129] = matmul(lhsT=P^T slice, rhs=V_aug) accumulated over
  d-blocks, where V_aug carries a ones column so the softmax denominator
  falls out of the same matmul. Normalize by 1/r per-partition, transpose O
  via TensorE, project back to [s, m] (ldweights shared across m-chunk
  pairs), evacuate on VectorE, one 1MB output DMA per 128-row block.
  The output projection is fused per s-block into the attention loop, and
  J=3's PV is interleaved into its score pass to fill the tail.
"""

import sys

sys.path.insert(0, "/opt/trn_rl_repo")

import math
from contextlib import ExitStack

import numpy as np
import ml_dtypes

import concourse.bass as bass
import concourse.tile as tile
from concourse.masks import make_identity
from concourse import bacc
from concourse import mybir
from concourse.bass_utils import run_bass_kernel_spmd

BF16 = ml_dtypes.bfloat16
T = 2048
C = 2048
HD = 128
NQH, NKVH = 16, 4
R = NQH // NKVH  # 4
ROPE_THETA = 10000.0
SOFTCAP = 50.0
NCORES = 8

F32 = mybir.dt.float32
BF = mybir.dt.bfloat16
AFT = mybir.ActivationFunctionType

TANH_SCALE = 1.0 / (math.sqrt(float(HD)) * SOFTCAP)
MASK_BIAS = -40.0  # added to tanh output; exp scale 50 -> -2000 in the exponent

_NC_CACHE = {}


def build_nc():
    if "nc" in _NC_CACHE:
        return _NC_CACHE["nc"]
    nc = bacc.Bacc(None, target_bir_lowering=False)
    xT = nc.dram_tensor("xT", [C, T], BF, kind="ExternalInput")
    wq = nc.dram_tensor("wq", [C, 2 * HD], BF, kind="ExternalInput")
    wk = nc.dram_tensor("wk", [C, HD], BF, kind="ExternalInput")
    wv = nc.dram_tensor("wv", [C, HD], BF, kind="ExternalInput")
    wo = nc.dram_tensor("wo", [2 * HD, C], BF, kind="ExternalInput")
    cosT = nc.dram_tensor("cosT", [HD, T], BF, kind="ExternalInput")
    sinT = nc.dram_tensor("sinT", [HD, T], F32, kind="ExternalInput")
    rmT = nc.dram_tensor("rmT", [HD, HD], BF, kind="ExternalInput")
    tri = nc.dram_tensor("tri", [HD, HD], F32, kind="ExternalInput")
    out = nc.dram_tensor("out", [T, C], F32, kind="ExternalOutput")

    NCH = C // 128  # 16 contraction chunks
    NSB = T // 128  # 16 s-blocks
    NJ = T // 512  # 4 s-chunks of 512

    with tile.TileContext(nc) as tc, ExitStack() as ctx:
        consts = ctx.enter_context(tc.tile_pool(name="consts", bufs=1))
        qkv = ctx.enter_context(tc.tile_pool(name="qkv", bufs=1))
        osmall = ctx.enter_context(tc.tile_pool(name="osmall", bufs=2))
        outsb = ctx.enter_context(tc.tile_pool(name="outsb", bufs=2))
        tpool = ctx.enter_context(tc.tile_pool(name="tpool", bufs=2))
        ptpool = []
        # PSUM budget (8 banks): proj 2 + sg 4 + o 1 + ot 1
        ps = ctx.enter_context(tc.tile_pool(name="ps", bufs=2, space="PSUM"))
        ps_o = ctx.enter_context(tc.tile_pool(name="ps_o", bufs=1, space="PSUM"))
        ps_sg = ctx.enter_context(tc.tile_pool(name="ps_sg", bufs=2, space="PSUM"))
        ps_ot = ctx.enter_context(tc.tile_pool(name="ps_ot", bufs=1, space="PSUM"))

        ident = consts.tile([128, 128], BF, tag="ident")
        make_identity(nc, ident)
        tri_sb = consts.tile([128, 128], F32, tag="tri")
        wo_sb = consts.tile([128, 2, C], BF, tag="wo")

        QT = qkv.tile([128, 2, T], BF, tag="QT")
        KT = qkv.tile([128, T], BF, tag="KT")
        Vaug = qkv.tile([128, NCH, 132], BF, tag="Vaug")
        OT = qkv.tile([128, 2, T], BF, tag="OT")
        nc.vector.memset(Vaug[:, :, 128:129], 1.0)

        pt_tiles = {}

        def attn_scores(J, i_lo=0, i_hi=None):
            n_i = 4 * J + 4
            if i_hi is None:
                i_hi = n_i
            if i_lo == 0:
                pool_ = qkv if J < 2 else ptpool[0]
                PT = pool_.tile(
                    [128, 2, n_i, 512], BF, tag=f"pt{min(J, 2)}", name=f"PT{J}"
                )
                pt_tiles[J] = PT
            else:
                PT = pt_tiles[J]
            for i in range(i_lo, i_hi):
                b = i - 4 * J
                c0 = 256 if b >= 2 else 0  # cols below are never consumed
                csl = slice(c0, 512)
                sg = ps_sg.tile([128, 2, 512], F32, tag="sg")
                for h in range(2):
                    nc.tensor.matmul(
                        sg[:, h, csl],
                        KT[:, i * 128:(i + 1) * 128],
                        QT[:, h, J * 512 + c0:(J + 1) * 512],
                        start=True, stop=True,
                    )
                tt = tpool.tile([128, 2, 512], F32, tag="t")
                nc.scalar.activation(
                    tt[:, :, csl], sg[:, :, csl], AFT.Tanh, scale=TANH_SCALE
                )
                if b >= 0:  # diagonal block: apply triangular mask bias
                    dsl = slice(b * 128, (b + 1) * 128)
                    for h in range(2):
                        nc.vector.tensor_add(tt[:, h, dsl], tt[:, h, dsl], tri_sb)
                nc.scalar.activation(
                    PT[:, :, i, csl], tt[:, :, csl], AFT.Exp, scale=SOFTCAP
                )

        def attn_pv_out(J, sbs=(0, 1, 2, 3), pop=True):
            PT = pt_tiles.pop(J) if pop else pt_tiles[J]
            for sb_ in sbs:
                j = 4 * J + sb_
                for h in range(2):
                    po = ps_o.tile([128, 129], F32, tag="o", name=f"po_{J}_{sb_}_{h}")
                    for i in range(j + 1):
                        nc.tensor.matmul(
                            po,
                            PT[:, h, i, sb_ * 128:(sb_ + 1) * 128],
                            Vaug[:, i, 0:129],
                            start=(i == 0), stop=(i == j),
                        )
                    rinv = osmall.tile([128, 1], F32, tag="rinv")
                    nc.vector.reciprocal(rinv, po[:, 128:129])
                    on = osmall.tile([128, 128], BF, tag="on")
                    nc.vector.tensor_scalar_mul(on, po[:, 0:128], rinv)
                    pot = ps_ot.tile([128, 128], BF, tag="ot")
                    nc.tensor.transpose(pot, on, ident)
                    nc.vector.tensor_copy(OT[:, h, j * 128:(j + 1) * 128], pot)
                # fused output projection for this s-block; ldweights of
                # OT[h] shared across an m-chunk pair; one 1MB DMA per block
                ob = outsb.tile([128, T], F32, tag="ob")
                for mg in range(2):
                    pp = [ps.tile([128, 512], F32, tag="proj", name=f"po{j}_{mg}{_i}")
                          for _i in range(2)]
                    for h in range(2):
                        for pi in range(2):
                            mch = 2 * mg + pi
                            nc.tensor.matmul(
                                pp[pi],
                                OT[:, h, j * 128:(j + 1) * 128],
                                wo_sb[:, h, mch * 512:(mch + 1) * 512],
                                start=(h == 0), stop=(h == 1),
                            )
                    for pi in range(2):
                        mch = 2 * mg + pi
                        nc.vector.tensor_copy(
                            ob[:, mch * 512:(mch + 1) * 512], pp[pi]
                        )
                nc.sync.dma_start(out=out[j * 128:(j + 1) * 128, :], in_=ob)

        with tc.tile_pool(name="ph1", bufs=1) as ph1, \
             tc.tile_pool(name="work", bufs=3) as work, \
             tc.tile_pool(name="ropet", bufs=2) as ropet:
            rm_sb = ph1.tile([128, 128], BF, tag="rm")
            cos_sb = ph1.tile([128, T], BF, tag="cos")
            sin_sb = ph1.tile([128, T], F32, tag="sin")
            wq_sb = ph1.tile([128, NCH, 2 * HD], BF, tag="wq")
            wk_sb = ph1.tile([128, NCH, HD], BF, tag="wk")
            wv_sb = ph1.tile([128, NCH, HD], BF, tag="wv")
            x_sb = ph1.tile([128, NCH, T], BF, tag="x")
            # batched DMAs (DMA_DIRECT2D issue is ~600ns each on Sync):
            # weights first (small), x staggered so K matmuls start early.
            def dma_chunks(dst, src, lo, hi):
                nc.sync.dma_start(
                    out=dst[:, lo:hi, :],
                    in_=src.rearrange("(c p) s -> p c s", p=128)[:, lo:hi, :],
                )
            dma_chunks(x_sb, xT, 0, 2)
            dma_chunks(wk_sb, wk, 0, NCH)
            dma_chunks(x_sb, xT, 2, 4)
            dma_chunks(wq_sb, wq, 0, NCH)
            dma_chunks(x_sb, xT, 4, 8)
            nc.sync.dma_start(out=rm_sb, in_=rmT[:, :])
            nc.sync.dma_start(out=cos_sb, in_=cosT[:, :])
            nc.sync.dma_start(out=sin_sb, in_=sinT[:, :])
            dma_chunks(x_sb, xT, 8, 12)
            dma_chunks(x_sb, xT, 12, 16)
            dma_chunks(wv_sb, wv, 0, NCH)
            nc.sync.dma_start(out=tri_sb, in_=tri[:, :])
            for h in range(2):
                nc.sync.dma_start(out=wo_sb[:, h, :], in_=wo[h * 128:(h + 1) * 128, :])

            def rope_chunk(z, ch, dst):
                sl = slice(ch * 512, (ch + 1) * 512)
                pr = ps.tile([128, 512], F32, tag="proj")
                nc.tensor.matmul(pr, rm_sb, z, start=True, stop=True)
                m2 = ropet.tile([128, 512], F32, tag="m2")
                nc.vector.tensor_mul(m2, pr, sin_sb[:, sl])
                m1 = ropet.tile([128, 512], F32, tag="m1")
                nc.vector.tensor_mul(m1, z, cos_sb[:, sl])
                nc.vector.tensor_add(dst[:, sl], m1, m2)

            def proj_chunk(w_slice_fn, ch, dst):
                sl = slice(ch * 512, (ch + 1) * 512)
                p = ps.tile([128, 512], F32, tag="proj")
                for c in range(NCH):
                    nc.tensor.matmul(
                        p, w_slice_fn(c), x_sb[:, c, sl],
                        start=(c == 0), stop=(c == NCH - 1),
                    )
                z = work.tile([128, 512], BF, tag="z")
                nc.scalar.copy(z, p)
                rope_chunk(z, ch, dst)

            def v_chunk(ch):
                sl = slice(ch * 512, (ch + 1) * 512)
                p = ps.tile([128, 512], F32, tag="proj")
                for c in range(NCH):
                    nc.tensor.matmul(
                        p, wv_sb[:, c, :], x_sb[:, c, sl],
                        start=(c == 0), stop=(c == NCH - 1),
                    )
                z = work.tile([128, 512], BF, tag="z")
                nc.scalar.copy(z, p)
                for b in range(4):
                    dt = 4 * ch + b
                    pv = ps_ot.tile([128, 128], BF, tag="ot")
                    nc.tensor.transpose(pv, z[:, b * 128:(b + 1) * 128], ident)
                    nc.vector.tensor_copy(Vaug[:, dt, 0:128], pv)

            # K: c-outer accumulation (borrows the two sg slots) -- matmuls
            # start with the first streamed x quarter, ldweights amortized.
            # in-stream: first two 512-chunks of K, Q0, Q1 accumulate
            # c-outer while x streams in (6 matmuls per x chunk ~ arrival
            # rate); remaining chunks + V run ch-outer afterwards.
            k0 = work.tile([128, T], BF, tag="zk", bufs=3)
            q0 = work.tile([128, T], BF, tag="zk", bufs=3)
            q1 = work.tile([128, T], BF, tag="zk", bufs=3)
            pkA = ps_sg.tile([128, 2, 512], F32, tag="sg", name="pkA")
            pq0A = ps_sg.tile([128, 2, 512], F32, tag="sg", name="pq0A")
            pq1A = [ps.tile([128, 512], F32, tag="proj", name=f"pq1A{_i}")
                    for _i in range(2)]
            for c in range(NCH):
                for ch in range(2):
                    nc.tensor.matmul(
                        pkA[:, ch, :], wk_sb[:, c, :],
                        x_sb[:, c, ch * 512:(ch + 1) * 512],
                        start=(c == 0), stop=(c == NCH - 1),
                    )
                for ch in range(2):
                    nc.tensor.matmul(
                        pq0A[:, ch, :], wq_sb[:, c, 0:HD],
                        x_sb[:, c, ch * 512:(ch + 1) * 512],
                        start=(c == 0), stop=(c == NCH - 1),
                    )
                for ch in range(2):
                    nc.tensor.matmul(
                        pq1A[ch], wq_sb[:, c, HD:2 * HD],
                        x_sb[:, c, ch * 512:(ch + 1) * 512],
                        start=(c == 0), stop=(c == NCH - 1),
                    )
            nc.scalar.copy(
                k0[:, 0:1024].rearrange("p (a b) -> p a b", a=2), pkA)
            nc.scalar.copy(
                q0[:, 0:1024].rearrange("p (a b) -> p a b", a=2), pq0A)
            for ch in range(2):
                nc.scalar.copy(q1[:, ch * 512:(ch + 1) * 512], pq1A[ch])
            for ch in range(2):
                rope_chunk(k0[:, ch * 512:(ch + 1) * 512], ch, KT)
                rope_chunk(q0[:, ch * 512:(ch + 1) * 512], ch, QT[:, 0, :])
                rope_chunk(q1[:, ch * 512:(ch + 1) * 512], ch, QT[:, 1, :])

            attn_scores(0)
            for ch in range(2, NJ):
                proj_chunk(lambda c: wk_sb[:, c, :], ch, KT)
            attn_scores(1)
            for ch in range(2, NJ):
                proj_chunk(lambda c: wq_sb[:, c, 0:HD], ch, QT[:, 0, :])
                proj_chunk(lambda c: wq_sb[:, c, HD:2 * HD], ch, QT[:, 1, :])
            for ch in range(NJ):
                v_chunk(ch)

        ptpool.append(ctx.enter_context(tc.tile_pool(name="ptpool", bufs=2)))
        attn_pv_out(0)
        attn_scores(2)
        attn_pv_out(1)
        attn_scores(3, 0, 13)
        attn_pv_out(2)
        attn_pv_out(3, sbs=(0,), pop=False)
        attn_scores(3, 13, 14)
        attn_pv_out(3, sbs=(1,), pop=False)
        attn_scores(3, 14, 15)
        attn_pv_out(3, sbs=(2,), pop=False)
        attn_scores(3, 15, 16)
        attn_pv_out(3, sbs=(3,))

    nc.finalize()
    _NC_CACHE["nc"] = nc
    return nc


def _rope_tables():
    fraction = np.arange(0, HD, 2, dtype=np.float64) / HD
    timescale = ROPE_THETA ** fraction
    inv = 1.0 / timescale
    sin_inp = np.outer(np.arange(T, dtype=np.float64), inv)
    sin_inp = np.concatenate([sin_inp, sin_inp], axis=-1)  # [T, HD]
    sin = np.sin(sin_inp).astype(np.float32)
    cos = np.cos(sin_inp).astype(np.float32)
    return cos.T.copy(), sin.T.copy()  # [HD, T]


def _numpy_fallback(x, mask, q_kernel, k_kernel, v_kernel, out_kernel):
    # generic-mask reference path (host, f32) - only used if the mask is not
    # the standard causal mask.
    b, t, c = x.shape
    q = np.einsum("bsm,mrhk->brhsk", x, q_kernel, optimize=True)
    k = np.einsum("bdm,mhk->bhdk", x, k_kernel, optimize=True)
    v = np.einsum("bdm,mhv->bhdv", x, v_kernel, optimize=True)
    cosT, sinT = _rope_tables()
    cos, sin = cosT.T, sinT.T  # [T, HD]

    def rot(z):
        z1, z2 = np.split(z, 2, axis=-1)
        return np.concatenate([-z2, z1], axis=-1)

    q = q * cos[None, None, None] + rot(q) * sin[None, None, None]
    k = k * cos[None, None] + rot(k) * sin[None, None]
    s = np.einsum("brhsk,bhdk->brhsd", q, k, optimize=True) / np.sqrt(np.float32(HD))
    s = np.tanh(s / SOFTCAP) * SOFTCAP
    m = mask[:, None]  # [B,1,1,T,T]
    s = np.where(m, s, -np.inf)
    s = s - s.max(axis=-1, keepdims=True)
    e = np.exp(s)
    p = e / e.sum(axis=-1, keepdims=True)
    p = np.where(m, p, 0.0)
    qkv = np.einsum("brhsd,bhdv->brhsv", p, v, optimize=True)
    return np.einsum("brhsv,rhvm->bsm", qkv, out_kernel, optimize=True).astype(np.float32)


def kernel(x, mask, q_kernel, k_kernel, v_kernel, out_kernel, _trace=False):
    x = np.asarray(x)
    mask = np.asarray(mask)
    causal = bool(
        np.array_equal(mask[0, 0], np.tril(np.ones((T, T), dtype=bool)))
    )
    if not causal:
        return _numpy_fallback(x, mask, q_kernel, k_kernel, v_kernel, out_kernel)

    q_kernel = np.asarray(q_kernel, dtype=np.float32)
    k_kernel = np.asarray(k_kernel, dtype=np.float32)
    v_kernel = np.asarray(v_kernel, dtype=np.float32)
    out_kernel = np.asarray(out_kernel, dtype=np.float32)

    xT = np.ascontiguousarray(x[0].T).astype(BF16)
    cosT, sinT = _rope_tables()
    cosT_bf = cosT.astype(BF16)
    rm = np.zeros((HD, HD), dtype=np.float32)
    for kk in range(HD // 2):
        rm[kk, kk + HD // 2] = -1.0
    for kk in range(HD // 2, HD):
        rm[kk, kk - HD // 2] = 1.0
    rmT = np.ascontiguousarray(rm.T).astype(BF16)
    dl = np.arange(128)[:, None]
    sl = np.arange(128)[None, :]
    tri = np.where(dl <= sl, 0.0, MASK_BIAS).astype(np.float32)

    in_maps = []
    for core in range(NCORES):
        h = core // 2
        r0 = (core % 2) * 2
        wq_c = np.ascontiguousarray(
            q_kernel[:, r0:r0 + 2, h, :].reshape(C, 2 * HD)
        ).astype(BF16)
        wk_c = np.ascontiguousarray(k_kernel[:, h, :]).astype(BF16)
        wv_c = np.ascontiguousarray(v_kernel[:, h, :]).astype(BF16)
        wo_c = np.ascontiguousarray(
            out_kernel[r0:r0 + 2, h, :, :].reshape(2 * HD, C)
        ).astype(BF16)
        in_maps.append({
            "xT": xT, "wq": wq_c, "wk": wk_c, "wv": wv_c, "wo": wo_c,
            "cosT": cosT_bf, "sinT": sinT, "rmT": rmT, "tri": tri,
        })

    nc = build_nc()
    res = run_bass_kernel_spmd(
        nc, in_maps, core_ids=list(range(NCORES)), trace=_trace
    )
    total = np.zeros((T, C), dtype=np.float32)
    for om in res.results:
        total += om["out"]
    out = total[None]
    if _trace:
        return out, res
    return out


# revision 30
# speedup vs baseline: 1.4403x; 1.0287x over previous
"""GQA attention (B=1, T=2048, C=2048, 16 Q heads / 4 KV heads, head_dim=128)
with RoPE, logit softcap 50, causal mask, softmax, output projection.

Sharding: 16 Q-heads over 8 NeuronCores (2 Q-heads + their single KV head per
core, tensor-parallel over the kv-head axis per the sharding hint). Each core
computes its partial output projection over its 2 heads; the host sums the 8
partials (the post-projection all-reduce).

Per-core device kernel (all matmuls bf16 with f32 PSUM accumulation):
  xT [C, T] bf16 resident in SBUF; K^T/Q^T projected in [head_dim, seq]
  layout via matmul(lhsT=W chunk, rhs=xT chunk). The K projection (plus the
  first half of Q0) accumulates c-outer while x streams in, so PE starts at
  the first 2MB DMA quarter. DMAs are batched (issue costs ~600ns each).
  RoPE in [k, s] layout: rot(q) = Rm @ q via a sign-permutation matmul, then
  q*cosT + rot*sinT on VectorE.
  S^T [d, s] = matmul(lhsT=K^T block, rhs=Q^T chunk): the post-softmax matrix
  is then already the PV lhsT -- no transpose of P. Both heads share each
  K-block ldweights. 512-granular causal: blocks entirely above the diagonal
  are skipped; partially-masked right-of-diagonal columns are trimmed.
  Softcap bounds logits to +-50, so softmax needs no max pass: tanh on
  ScalarE (scale 1/(50*sqrt(128))), triangular -40 bias on diagonal blocks
  (DVE), exp on ScalarE (scale 50; tanh+exp share one ACT table set).
  PV: O_aug[s, 129] = matmul(lhsT=P^T slice, rhs=V_aug) accumulated over
  d-blocks, where V_aug carries a ones column so the softmax denominator
  falls out of the same matmul. Normalize by 1/r per-partition, transpose O
  via TensorE, project back to [s, m] (ldweights shared across m-chunk
  pairs), evacuate on VectorE, one 1MB output DMA per 128-row block.
  The output projection is fused per s-block into the attention loop, and
  J=3's PV is interleaved into its score pass to fill the tail.
"""

import sys

sys.path.insert(0, "/opt/trn_rl_repo")

import math
from contextlib import ExitStack

import numpy as np
import ml_dtypes

import concourse.bass as bass
import concourse.tile as tile
from concourse.masks import make_identity
from concourse import bacc
from concourse import mybir
from concourse.bass_utils import run_bass_kernel_spmd

BF16 = ml_dtypes.bfloat16
T = 2048
C = 2048
HD = 128
NQH, NKVH = 16, 4
R = NQH // NKVH  # 4
ROPE_THETA = 10000.0
SOFTCAP = 50.0
NCORES = 8

F32 = mybir.dt.float32
BF = mybir.dt.bfloat16
AFT = mybir.ActivationFunctionType

TANH_SCALE = 1.0 / (math.sqrt(float(HD)) * SOFTCAP)
MASK_BIAS = -40.0  # added to tanh output; exp scale 50 -> -2000 in the exponent

_NC_CACHE = {}


def build_nc():
    if "nc" in _NC_CACHE:
        return _NC_CACHE["nc"]
    nc = bacc.Bacc(None, target_bir_lowering=False)
    xT = nc.dram_tensor("xT", [C, T], BF, kind="ExternalInput")
    wq = nc.dram_tensor("wq", [C, 2 * HD], BF, kind="ExternalInput")
    wk = nc.dram_tensor("wk", [C, HD], BF, kind="ExternalInput")
    wv = nc.dram_tensor("wv", [C, HD], BF, kind="ExternalInput")
    wo = nc.dram_tensor("wo", [2 * HD, C], BF, kind="ExternalInput")
    cosT = nc.dram_tensor("cosT", [HD, T], BF, kind="ExternalInput")
    sinT = nc.dram_tensor("sinT", [HD, T], F32, kind="ExternalInput")
    rmT = nc.dram_tensor("rmT", [HD, HD], BF, kind="ExternalInput")
    tri = nc.dram_tensor("tri", [HD, HD], F32, kind="ExternalInput")
    out = nc.dram_tensor("out", [T, C], F32, kind="ExternalOutput")

    NCH = C // 128  # 16 contraction chunks
    NSB = T // 128  # 16 s-blocks
    NJ = T // 512  # 4 s-chunks of 512

    with tile.TileContext(nc) as tc, ExitStack() as ctx:
        consts = ctx.enter_context(tc.tile_pool(name="consts", bufs=1))
        qkv = ctx.enter_context(tc.tile_pool(name="qkv", bufs=1))
        osmall = ctx.enter_context(tc.tile_pool(name="osmall", bufs=2))
        outsb = ctx.enter_context(tc.tile_pool(name="outsb", bufs=2))
        tpool = ctx.enter_context(tc.tile_pool(name="tpool", bufs=2))
        ptpool = []
        # PSUM budget (8 banks): proj 2 + sg 4 + o 1 + ot 1
        ps = ctx.enter_context(tc.tile_pool(name="ps", bufs=2, space="PSUM"))
        ps_o = ctx.enter_context(tc.tile_pool(name="ps_o", bufs=1, space="PSUM"))
        ps_sg = ctx.enter_context(tc.tile_pool(name="ps_sg", bufs=2, space="PSUM"))
        ps_ot = ctx.enter_context(tc.tile_pool(name="ps_ot", bufs=1, space="PSUM"))

        ident = consts.tile([128, 128], BF, tag="ident")
        make_identity(nc, ident)
        tri_sb = consts.tile([128, 128], F32, tag="tri")
        wo_sb = consts.tile([128, 2, C], BF, tag="wo")

        QT = qkv.tile([128, 2, T], BF, tag="QT")
        KT = qkv.tile([128, T], BF, tag="KT")
        Vaug = qkv.tile([128, NCH, 132], BF, tag="Vaug")
        OT = qkv.tile([128, 2, T], BF, tag="OT")
        nc.vector.memset(Vaug[:, :, 128:129], 1.0)

        pt_tiles = {}

        def attn_scores(J, i_lo=0, i_hi=None):
            n_i = 4 * J + 4
            if i_hi is None:
                i_hi = n_i
            assert i_lo % 2 == 0 and i_hi % 2 == 0
            if i_lo == 0:
                pool_ = qkv if J < 2 else ptpool[0]
                PT = pool_.tile(
                    [128, 2, n_i, 512], BF, tag=f"pt{min(J, 2)}", name=f"PT{J}"
                )
                pt_tiles[J] = PT
            else:
                PT = pt_tiles[J]
            tt = None
            for i in range(i_lo, i_hi):
                b = i - 4 * J
                c0 = 256 if b >= 2 else 0  # cols below are never consumed
                csl = slice(c0, 512)
                sg = ps_sg.tile([128, 2, 512], F32, tag="sg")
                for h in range(2):
                    nc.tensor.matmul(
                        sg[:, h, csl],
                        KT[:, i * 128:(i + 1) * 128],
                        QT[:, h, J * 512 + c0:(J + 1) * 512],
                        start=True, stop=True,
                    )
                if i % 2 == 0:
                    tt = tpool.tile([128, 2, 2, 512], F32, tag="t")
                    pair_c0 = c0
                nc.scalar.activation(
                    tt[:, :, i % 2, csl], sg[:, :, csl], AFT.Tanh,
                    scale=TANH_SCALE,
                )
                if b >= 0:  # diagonal block: apply triangular mask bias
                    dsl = slice(b * 128, (b + 1) * 128)
                    for h in range(2):
                        nc.vector.tensor_add(
                            tt[:, h, i % 2, dsl], tt[:, h, i % 2, dsl], tri_sb
                        )
                if i % 2 == 1:
                    # one exp per pair of d-blocks (the 352-cycle ACT
                    # instruction overhead is paid once, not twice)
                    psl = slice(pair_c0, 512)
                    nc.scalar.activation(
                        PT[:, :, i - 1:i + 1, psl], tt[:, :, :, psl],
                        AFT.Exp, scale=SOFTCAP,
                    )

        def attn_pv_out(J, sbs=(0, 1, 2, 3), pop=True):
            PT = pt_tiles.pop(J) if pop else pt_tiles[J]
            for sb_ in sbs:
                j = 4 * J + sb_
                for h in range(2):
                    po = ps_o.tile([128, 129], F32, tag="o", name=f"po_{J}_{sb_}_{h}")
                    for i in range(j + 1):
                        nc.tensor.matmul(
                            po,
                            PT[:, h, i, sb_ * 128:(sb_ + 1) * 128],
                            Vaug[:, i, 0:129],
                            start=(i == 0), stop=(i == j),
                        )
                    rinv = osmall.tile([128, 1], F32, tag="rinv")
                    nc.vector.reciprocal(rinv, po[:, 128:129])
                    on = osmall.tile([128, 128], BF, tag="on")
                    nc.vector.tensor_scalar_mul(on, po[:, 0:128], rinv)
                    pot = ps_ot.tile([128, 128], BF, tag="ot")
                    nc.tensor.transpose(pot, on, ident)
                    nc.vector.tensor_copy(OT[:, h, j * 128:(j + 1) * 128], pot)
                # fused output projection for this s-block; ldweights of
                # OT[h] shared across an m-chunk pair; one 1MB DMA per block
                ob = outsb.tile([128, T], F32, tag="ob")
                for mg in range(2):
                    pp = [ps.tile([128, 512], F32, tag="proj", name=f"po{j}_{mg}{_i}")
                          for _i in range(2)]
                    for h in range(2):
                        for pi in range(2):
                            mch = 2 * mg + pi
                            nc.tensor.matmul(
                                pp[pi],
                                OT[:, h, j * 128:(j + 1) * 128],
                                wo_sb[:, h, mch * 512:(mch + 1) * 512],
                                start=(h == 0), stop=(h == 1),
                            )
                    for pi in range(2):
                        mch = 2 * mg + pi
                        nc.vector.tensor_copy(
                            ob[:, mch * 512:(mch + 1) * 512], pp[pi]
                        )
                nc.sync.dma_start(out=out[j * 128:(j + 1) * 128, :], in_=ob)

        with tc.tile_pool(name="ph1", bufs=1) as ph1, \
             tc.tile_pool(name="work", bufs=3) as work, \
             tc.tile_pool(name="ropet", bufs=2) as ropet:
            rm_sb = ph1.tile([128, 128], BF, tag="rm")
            cos_sb = ph1.tile([128, T], BF, tag="cos")
            sin_sb = ph1.tile([128, T], F32, tag="sin")
            wq_sb = ph1.tile([128, NCH, 2 * HD], BF, tag="wq")
            wk_sb = ph1.tile([128, NCH, HD], BF, tag="wk")
            wv_sb = ph1.tile([128, NCH, HD], BF, tag="wv")
            x_sb = ph1.tile([128, NCH, T], BF, tag="x")
            # batched DMAs (DMA_DIRECT2D issue is ~600ns each on Sync):
            # weights first (small), x staggered so K matmuls start early.
            def dma_chunks(dst, src, lo, hi):
                nc.sync.dma_start(
                    out=dst[:, lo:hi, :],
                    in_=src.rearrange("(c p) s -> p c s", p=128)[:, lo:hi, :],
                )
            dma_chunks(x_sb, xT, 0, 1)
            dma_chunks(wk_sb, wk, 0, NCH)
            dma_chunks(x_sb, xT, 1, 2)
            dma_chunks(x_sb, xT, 2, 4)
            dma_chunks(wq_sb, wq, 0, NCH)
            dma_chunks(x_sb, xT, 4, 8)
            nc.sync.dma_start(out=rm_sb, in_=rmT[:, :])
            nc.sync.dma_start(out=cos_sb, in_=cosT[:, :])
            nc.sync.dma_start(out=sin_sb, in_=sinT[:, :])
            dma_chunks(x_sb, xT, 8, 12)
            dma_chunks(x_sb, xT, 12, 16)
            dma_chunks(wv_sb, wv, 0, NCH)
            nc.sync.dma_start(out=tri_sb, in_=tri[:, :])
            for h in range(2):
                nc.sync.dma_start(out=wo_sb[:, h, :], in_=wo[h * 128:(h + 1) * 128, :])

            def rope_chunk(z, ch, dst):
                sl = slice(ch * 512, (ch + 1) * 512)
                pr = ps.tile([128, 512], F32, tag="proj")
                nc.tensor.matmul(pr, rm_sb, z, start=True, stop=True)
                m2 = ropet.tile([128, 512], F32, tag="m2")
                nc.vector.tensor_mul(m2, pr, sin_sb[:, sl])
                m1 = ropet.tile([128, 512], F32, tag="m1")
                nc.vector.tensor_mul(m1, z, cos_sb[:, sl])
                nc.vector.tensor_add(dst[:, sl], m1, m2)

            def proj_chunk(w_slice_fn, ch, dst):
                sl = slice(ch * 512, (ch + 1) * 512)
                p = ps.tile([128, 512], F32, tag="proj")
                for c in range(NCH):
                    nc.tensor.matmul(
                        p, w_slice_fn(c), x_sb[:, c, sl],
                        start=(c == 0), stop=(c == NCH - 1),
                    )
                z = work.tile([128, 512], BF, tag="z")
                nc.scalar.copy(z, p)
                rope_chunk(z, ch, dst)

            def v_chunk(ch):
                sl = slice(ch * 512, (ch + 1) * 512)
                p = ps.tile([128, 512], F32, tag="proj")
                for c in range(NCH):
                    nc.tensor.matmul(
                        p, wv_sb[:, c, :], x_sb[:, c, sl],
                        start=(c == 0), stop=(c == NCH - 1),
                    )
                z = work.tile([128, 512], BF, tag="z")
                nc.scalar.copy(z, p)
                for b in range(4):
                    dt = 4 * ch + b
                    pv = ps_ot.tile([128, 128], BF, tag="ot")
                    nc.tensor.transpose(pv, z[:, b * 128:(b + 1) * 128], ident)
                    nc.vector.tensor_copy(Vaug[:, dt, 0:128], pv)

            # K: c-outer accumulation (borrows the two sg slots) -- matmuls
            # start with the first streamed x quarter, ldweights amortized.
            # in-stream: first two 512-chunks of K, Q0, Q1 accumulate
            # c-outer while x streams in (6 matmuls per x chunk ~ arrival
            # rate); remaining chunks + V run ch-outer afterwards.
            k0 = work.tile([128, T], BF, tag="zk", bufs=3)
            q0 = work.tile([128, T], BF, tag="zk", bufs=3)
            q1 = work.tile([128, T], BF, tag="zk", bufs=3)
            pkA = ps_sg.tile([128, 2, 512], F32, tag="sg", name="pkA")
            pq0A = ps_sg.tile([128, 2, 512], F32, tag="sg", name="pq0A")
            pq1A = [ps.tile([128, 512], F32, tag="proj", name=f"pq1A{_i}")
                    for _i in range(2)]
            for c in range(NCH):
                for ch in range(2):
                    nc.tensor.matmul(
                        pkA[:, ch, :], wk_sb[:, c, :],
                        x_sb[:, c, ch * 512:(ch + 1) * 512],
                        start=(c == 0), stop=(c == NCH - 1),
                    )
                for ch in range(2):
                    nc.tensor.matmul(
                        pq0A[:, ch, :], wq_sb[:, c, 0:HD],
                        x_sb[:, c, ch * 512:(ch + 1) * 512],
                        start=(c == 0), stop=(c == NCH - 1),
                    )
                for ch in range(2):
                    nc.tensor.matmul(
                        pq1A[ch], wq_sb[:, c, HD:2 * HD],
                        x_sb[:, c, ch * 512:(ch + 1) * 512],
                        start=(c == 0), stop=(c == NCH - 1),
                    )
            nc.scalar.copy(
                k0[:, 0:1024].rearrange("p (a b) -> p a b", a=2), pkA)
            nc.scalar.copy(
                q0[:, 0:1024].rearrange("p (a b) -> p a b", a=2), pq0A)
            for ch in range(2):
                nc.scalar.copy(q1[:, ch * 512:(ch + 1) * 512], pq1A[ch])
            for ch in range(2):
                rope_chunk(k0[:, ch * 512:(ch + 1) * 512], ch, KT)
                rope_chunk(q0[:, ch * 512:(ch + 1) * 512], ch, QT[:, 0, :])
                rope_chunk(q1[:, ch * 512:(ch + 1) * 512], ch, QT[:, 1, :])

            attn_scores(0)
            for ch in range(2, NJ):
                proj_chunk(lambda c: wk_sb[:, c, :], ch, KT)
            attn_scores(1)
            for ch in range(2, NJ):
                proj_chunk(lambda c: wq_sb[:, c, 0:HD], ch, QT[:, 0, :])
                proj_chunk(lambda c: wq_sb[:, c, HD:2 * HD], ch, QT[:, 1, :])
            for ch in range(NJ):
                v_chunk(ch)

        ptpool.append(ctx.enter_context(tc.tile_pool(name="ptpool", bufs=2)))
        attn_pv_out(0)
        attn_scores(2)
        attn_pv_out(1)
        attn_scores(3, 0, 14)
        attn_pv_out(2)
        attn_pv_out(3, sbs=(0, 1), pop=False)
        attn_scores(3, 14, 16)
        attn_pv_out(3, sbs=(2, 3))

    nc.finalize()
    _NC_CACHE["nc"] = nc
    return nc


def _rope_tables():
    fraction = np.arange(0, HD, 2, dtype=np.float64) / HD
    timescale = ROPE_THETA ** fraction
    inv = 1.0 / timescale
    sin_inp = np.outer(np.arange(T, dtype=np.float64), inv)
    sin_inp = np.concatenate([sin_inp, sin_inp], axis=-1)  # [T, HD]
    sin = np.sin(sin_inp).astype(np.float32)
    cos = np.cos(sin_inp).astype(np.float32)
    return cos.T.copy(), sin.T.copy()  # [HD, T]


def _numpy_fallback(x, mask, q_kernel, k_kernel, v_kernel, out_kernel):
    # generic-mask reference path (host, f32) - only used if the mask is not
    # the standard causal mask.
    b, t, c = x.shape
    q = np.einsum("bsm,mrhk->brhsk", x, q_kernel, optimize=True)
    k = np.einsum("bdm,mhk->bhdk", x, k_kernel, optimize=True)
    v = np.einsum("bdm,mhv->bhdv", x, v_kernel, optimize=True)
    cosT, sinT = _rope_tables()
    cos, sin = cosT.T, sinT.T  # [T, HD]

    def rot(z):
        z1, z2 = np.split(z, 2, axis=-1)
        return np.concatenate([-z2, z1], axis=-1)

    q = q * cos[None, None, None] + rot(q) * sin[None, None, None]
    k = k * cos[None, None] + rot(k) * sin[None, None]
    s = np.einsum("brhsk,bhdk->brhsd", q, k, optimize=True) / np.sqrt(np.float32(HD))
    s = np.tanh(s / SOFTCAP) * SOFTCAP
    m = mask[:, None]  # [B,1,1,T,T]
    s = np.where(m, s, -np.inf)
    s = s - s.max(axis=-1, keepdims=True)
    e = np.exp(s)
    p = e / e.sum(axis=-1, keepdims=True)
    p = np.where(m, p, 0.0)
    qkv = np.einsum("brhsd,bhdv->brhsv", p, v, optimize=True)
    return np.einsum("brhsv,rhvm->bsm", qkv, out_kernel, optimize=True).astype(np.float32)


def kernel(x, mask, q_kernel, k_kernel, v_kernel, out_kernel, _trace=False):
    x = np.asarray(x)
    mask = np.asarray(mask)
    causal = bool(
        np.array_equal(mask[0, 0], np.tril(np.ones((T, T), dtype=bool)))
    )
    if not causal:
        return _numpy_fallback(x, mask, q_kernel, k_kernel, v_kernel, out_kernel)

    q_kernel = np.asarray(q_kernel, dtype=np.float32)
    k_kernel = np.asarray(k_kernel, dtype=np.float32)
    v_kernel = np.asarray(v_kernel, dtype=np.float32)
    out_kernel = np.asarray(out_kernel, dtype=np.float32)

    xT = np.ascontiguousarray(x[0].T).astype(BF16)
    cosT, sinT = _rope_tables()
    cosT_bf = cosT.astype(BF16)
    rm = np.zeros((HD, HD), dtype=np.float32)
    for kk in range(HD // 2):
        rm[kk, kk + HD // 2] = -1.0
    for kk in range(HD // 2, HD):
        rm[kk, kk - HD // 2] = 1.0
    rmT = np.ascontiguousarray(rm.T).astype(BF16)
    dl = np.arange(128)[:, None]
    sl = np.arange(128)[None, :]
    tri = np.where(dl <= sl, 0.0, MASK_BIAS).astype(np.float32)

    in_maps = []
    for core in range(NCORES):
        h = core // 2
        r0 = (core % 2) * 2
        wq_c = np.ascontiguousarray(
            q_kernel[:, r0:r0 + 2, h, :].reshape(C, 2 * HD)
        ).astype(BF16)
        wk_c = np.ascontiguousarray(k_kernel[:, h, :]).astype(BF16)
        wv_c = np.ascontiguousarray(v_kernel[:, h, :]).astype(BF16)
        wo_c = np.ascontiguousarray(
            out_kernel[r0:r0 + 2, h, :, :].reshape(2 * HD, C)
        ).astype(BF16)
        in_maps.append({
            "xT": xT, "wq": wq_c, "wk": wk_c, "wv": wv_c, "wo": wo_c,
            "cosT": cosT_bf, "sinT": sinT, "rmT": rmT, "tri": tri,
        })

    nc = build_nc()
    res = run_bass_kernel_spmd(
        nc, in_maps, core_ids=list(range(NCORES)), trace=_trace
    )
    total = np.zeros((T, C), dtype=np.float32)
    for om in res.results:
        total += om["out"]
    out = total[None]
    if _trace:
        return out, res
    return out
